# revision 6
# baseline (speedup 1.0000x reference)
# GCN (2-layer GCNConv + linear head + softmax) on 8 Trainium2 NeuronCores.
#
# Math (matches PyG GCNConv with add_self_loops, symmetric norm):
#   A' = A + I,  deg = indegree(A') ,  dinv = deg^-1/2
#   out = softmax( relu( Ahat @ relu( Ahat @ (x W1) + b1 ) W2 + b2 ) Wout + bout )
#   with Ahat = D^-1/2 A' D^-1/2.
# We push dinv scalings onto node vectors:  Ahat h = dinv * (A'^T-gather-sum (dinv * h)).
#
# Distribution: nodes (rows) are range-sharded across 8 cores; edges are
# partitioned by destination core.  Per destination block of 128 nodes the
# incoming edge list is processed in chunks of 128 edges:
#   gather hs[src] rows with dma_gather (bf16 "pair" table [N/2, 128]: row r
#   holds nodes 2r and 2r+1, 256B — the minimum gather granule),
#   build a one-hot selection matrix S[e, dst] = (iota == dst_local[e]) on DVE,
#   and accumulate aggT[feat, dst] += msg[e, feat]^T via PE matmuls into PSUM.
# Dense phases (x@W1, r1@W2) are computed replicated on every core; the only
# cross-core exchange is an AllGather of r1 (relu of layer-1 output), split
# into pieces so it overlaps the layer-1 aggregation.
import math
import os
import sys
from dataclasses import dataclass, field

import numpy as np

sys.path.insert(0, "/opt/trn_rl_repo")
sys.path.insert(0, "/opt/pypackages")

import ml_dtypes

import concourse.bacc as bacc
import concourse.bass as bass
import concourse.mybir as mybir
import concourse.tile as tile

BF16 = mybir.dt.bfloat16
F32 = mybir.dt.float32
I16 = mybir.dt.int16
AF = mybir.ActivationFunctionType
OP = mybir.AluOpType

P = 128


@dataclass
class Cfg:
    n: int            # nodes (even, divisible by ncore)
    ncore: int
    feat: int         # 128
    hid: int          # 64
    ncls: int         # 16
    cp: int           # chunks per (block, parity) bucket  (uniform, SPMD)
    gg: int           # gather group size in chunks
    npieces: int

    ns: int = field(init=False)
    nblk: int = field(init=False)
    bw: list = field(init=False)        # block widths
    npair: int = field(init=False)
    nchunk: int = field(init=False)     # chunks per layer per core
    ngroups: int = field(init=False)
    nt_a: int = field(init=False)       # phase-A tiles over all nodes
    piece_blocks: list = field(init=False)   # list of (b0, b1)
    piece_rows: list = field(init=False)
    c1_tiles: list = field(init=False)  # flat [(piece, rank, t, node0, m)]

    def __post_init__(self):
        self.ns = self.n // self.ncore
        self.nblk = (self.ns + P - 1) // P
        self.bw = [min(P, self.ns - b * P) for b in range(self.nblk)]
        self.npair = self.n // 2
        self.nchunk = self.nblk * 2 * self.cp
        self.ngroups = (self.nchunk + self.gg - 1) // self.gg
        self.nt_a = (self.n + P - 1) // P
        npc = min(self.npieces, self.nblk)
        base, rem = divmod(self.nblk, npc)
        sizes = [base + (1 if i < rem else 0) for i in range(npc)]
        self.piece_blocks = []
        b0 = 0
        for s in sizes:
            self.piece_blocks.append((b0, b0 + s))
            b0 += s
        self.npieces = npc
        self.piece_rows = [
            sum(self.bw[b0:b1]) for (b0, b1) in self.piece_blocks
        ]
        self.c1_tiles = []
        for pi, (b0, b1) in enumerate(self.piece_blocks):
            prow0 = sum(self.bw[:b0])
            rows_p = self.piece_rows[pi]
            ntile = (rows_p + P - 1) // P
            for rb in range(self.ncore):
                for t in range(ntile):
                    m = min(P, rows_p - t * P)
                    node0 = rb * self.ns + prow0 + t * P
                    self.c1_tiles.append((pi, rb, t, node0, m))


def build_schedule(x, edge_index, W1, b1, W2, b2, Wout, bout, ncore=8,
                   npieces=4, gg=8):
    """Host-side preprocessing.  Returns (cfg, shared inputs, per-core inputs)."""
    n, feat = x.shape
    hid = W1.shape[1]
    ncls = Wout.shape[1]
    assert n % (2 * ncore) == 0

    src = np.concatenate([np.asarray(edge_index[0], dtype=np.int64),
                          np.arange(n, dtype=np.int64)]).astype(np.int32)
    dst = np.concatenate([np.asarray(edge_index[1], dtype=np.int64),
                          np.arange(n, dtype=np.int64)]).astype(np.int32)
    deg = np.bincount(dst, minlength=n).astype(np.float64)
    dinv = np.where(deg > 0, 1.0 / np.sqrt(np.maximum(deg, 1e-12)), 0.0)
    dinv = dinv.astype(np.float32)

    ns = n // ncore
    nblk = (ns + P - 1) // P
    core_of = dst // ns
    loc = dst % ns
    blk = loc >> 7
    dstl = (loc & 127).astype(np.float32)
    par = (src & 1).astype(np.int64)
    pidx = (src >> 1).astype(np.int32)

    nbucket = ncore * nblk * 2
    key = (core_of * nblk + blk) * 2 + par
    counts = np.bincount(key, minlength=nbucket)
    cp = int(math.ceil(counts.max() / P))

    cfg = Cfg(n=n, ncore=ncore, feat=feat, hid=hid, ncls=ncls, cp=cp,
              gg=gg, npieces=npieces)

    # Per-edge slot position inside the packed stream of its core.
    order = np.argsort(key, kind="stable")
    bucket_start = np.zeros(nbucket + 1, dtype=np.int64)
    np.cumsum(counts, out=bucket_start[1:])
    rank_in_bucket = np.empty(len(key), dtype=np.int64)
    ar = np.arange(len(key), dtype=np.int64)
    rank_in_bucket[order] = ar - bucket_start[key[order]]
    # slot base of bucket (within its core): (blk*2 + par) * cp * 128
    slot_base = ((blk * 2 + par) * cp) * P
    pos = slot_base + rank_in_bucket  # position within core stream

    tot = cfg.nchunk * P
    gidx_all = []
    dstl_all = []
    dinvT_all = []
    for c in range(ncore):
        sel = core_of == c
        stream_pidx = np.zeros(tot, dtype=np.int16)
        stream_dstl = np.full(tot, -1.0, dtype=np.float32)
        p_c = pos[sel]
        stream_pidx[p_c] = pidx[sel].astype(np.int16)
        stream_dstl[p_c] = dstl[sel]
        # wrapped index layout: idx at linear pos i -> [i % 16, i // 16],
        # replicated over the 8 groups of 16 partitions
        wrapped = stream_pidx.reshape(-1, 16).T  # [16, tot/16]
        gidx = np.tile(wrapped, (8, 1)).astype(np.int16)
        dstl_t = np.ascontiguousarray(stream_dstl.reshape(-1, P).T)
        gidx_all.append(np.ascontiguousarray(gidx))
        dstl_all.append(np.ascontiguousarray(dstl_t))
        dinvT_all.append(np.ascontiguousarray(
            np.broadcast_to(dinv[c * ns:(c + 1) * ns][None, :], (hid, ns))
        ).astype(np.float32))

    xs = (x.astype(np.float64) * dinv.astype(np.float64)[:, None])
    xsT = np.ascontiguousarray(xs.T.astype(ml_dtypes.bfloat16))

    dinvc1 = np.zeros((P, len(cfg.c1_tiles)), dtype=np.float32)
    for tc, (pi, rb, t, node0, m) in enumerate(cfg.c1_tiles):
        dinvc1[:m, tc] = dinv[node0:node0 + m]

    iota = np.tile(np.arange(P, dtype=np.float32), (P, 1)).astype(
        ml_dtypes.bfloat16)

    wout_aug = np.concatenate([Wout.astype(np.float32),
                               bout.astype(np.float32)[None, :]], axis=0)

    shared = {
        "xsT": xsT,
        "iota": np.ascontiguousarray(iota),
        "w1": np.ascontiguousarray(W1.astype(ml_dtypes.bfloat16)),
        "w2": np.ascontiguousarray(W2.astype(ml_dtypes.bfloat16)),
        "wout": np.ascontiguousarray(wout_aug.astype(ml_dtypes.bfloat16)),
        "b1": np.ascontiguousarray(b1.astype(np.float32)[:, None]),
        "b2": np.ascontiguousarray(b2.astype(np.float32)[:, None]),
        "dinvc1": dinvc1,
    }
    per_core = [
        {"gidx": gidx_all[c], "dstl": dstl_all[c], "dinvT": dinvT_all[c]}
        for c in range(ncore)
    ]
    return cfg, shared, per_core


def build_program(cfg: Cfg, debug=False):
    nc = bacc.Bacc("TRN2", debug=debug, enable_asserts=False,
                   target_bir_lowering=False, num_devices=cfg.ncore)
    hid, ncls = cfg.hid, cfg.ncls

    xsT = nc.dram_tensor("xsT", [cfg.feat, cfg.n], BF16, kind="ExternalInput")
    gidx = nc.dram_tensor("gidx", [P, cfg.nchunk * 8], I16, kind="ExternalInput")
    dstl = nc.dram_tensor("dstl", [P, cfg.nchunk], F32, kind="ExternalInput")
    iota = nc.dram_tensor("iota", [P, P], BF16, kind="ExternalInput")
    w1 = nc.dram_tensor("w1", [cfg.feat, hid], BF16, kind="ExternalInput")
    w2 = nc.dram_tensor("w2", [hid, hid], BF16, kind="ExternalInput")
    wout = nc.dram_tensor("wout", [hid + 1, ncls], BF16, kind="ExternalInput")
    b1 = nc.dram_tensor("b1", [hid, 1], F32, kind="ExternalInput")
    b2 = nc.dram_tensor("b2", [hid, 1], F32, kind="ExternalInput")
    dinvT = nc.dram_tensor("dinvT", [hid, cfg.ns], F32, kind="ExternalInput")
    dinvc1 = nc.dram_tensor("dinvc1", [P, len(cfg.c1_tiles)], F32,
                            kind="ExternalInput")
    out = nc.dram_tensor("out", [cfg.ns, ncls], F32, kind="ExternalOutput")

    hs1 = nc.dram_tensor("hs1", [cfg.npair, 2 * hid], BF16)
    hs2 = nc.dram_tensor("hs2", [cfg.npair, 2 * hid], BF16)
    r1loc = [nc.dram_tensor(f"r1loc{p}", [hid, cfg.piece_rows[p]], BF16)
             for p in range(cfg.npieces)]
    r1full = [nc.dram_tensor(f"r1full{p}",
                             [cfg.ncore, hid, cfg.piece_rows[p]], BF16,
                             addr_space="Shared")
              for p in range(cfg.npieces)]

    hs1w = hs1.ap().rearrange("a (b c) -> (a b) c", b=2, c=hid)
    hs2w = hs2.ap().rearrange("a (b c) -> (a b) c", b=2, c=hid)

    with tile.TileContext(nc) as tc:
        pools = []

        def mkpool(**kw):
            p = tc.alloc_tile_pool(**kw)
            pools.append(p)
            return p

        cpool = mkpool(name="const", bufs=1)
        iota_t = cpool.tile([P, P], BF16, tag="iota")
        w1_t = cpool.tile([cfg.feat, hid], BF16, tag="w1")
        w2_t = cpool.tile([hid, hid], BF16, tag="w2")
        wout_t = cpool.tile([hid + 1, ncls], BF16, tag="wout")
        b1_t = cpool.tile([hid, 1], F32, tag="b1")
        b2_t = cpool.tile([hid, 1], F32, tag="b2")
        dinvT_t = cpool.tile([hid, cfg.ns], F32, tag="dinvT")
        dinvc1_t = cpool.tile([P, len(cfg.c1_tiles)], F32, tag="dinvc1")
        gidx_t = cpool.tile([P, cfg.nchunk * 8], I16, tag="gidx")
        dstl_t = cpool.tile([P, cfg.nchunk], F32, tag="dstl")
        r1T_sb = cpool.tile([hid, cfg.ns], BF16, tag="r1T")

        for t_, d_ in ((iota_t, iota), (w1_t, w1), (w2_t, w2),
                       (wout_t, wout), (b1_t, b1), (b2_t, b2),
                       (dinvT_t, dinvT), (dinvc1_t, dinvc1),
                       (gidx_t, gidx), (dstl_t, dstl)):
            nc.sync.dma_start(out=t_[:], in_=d_[:, :])

        # pools
        SLAB = 16  # phase-A tiles per slab
        xpool = mkpool(name="xslab", bufs=2)
        stg_pool = mkpool(name="stg", bufs=4)
        s_pool = mkpool(name="smat", bufs=4)
        g_pool = mkpool(name="gbuf", bufs=3)
        r_pool = mkpool(name="rbuf", bufs=2)
        bpost_pool = mkpool(name="bpost", bufs=3)
        sm_pool = mkpool(name="smx", bufs=3)
        psA = mkpool(name="psA", bufs=2, space="PSUM")
        psT = mkpool(name="psT", bufs=2, space="PSUM")
        psL = mkpool(name="psL", bufs=2, space="PSUM")

        # ---------------- Phase A: hs1 = (dinv*x) @ W1, replicated ---------
        nslab = (cfg.nt_a + SLAB - 1) // SLAB
        for s in range(nslab):
            t0 = s * SLAB
            t1 = min(t0 + SLAB, cfg.nt_a)
            c0 = t0 * P
            c1 = min(t1 * P, cfg.n)
            xslab = xpool.tile([cfg.feat, SLAB * P], BF16, tag="xslab")
            nc.sync.dma_start(out=xslab[:, :c1 - c0], in_=xsT[:, c0:c1])
            for t in range(t0, t1):
                m = min(P, cfg.n - t * P)
                off = t * P - c0
                ps = psA.tile([P, hid], F32, tag="psA")
                nc.tensor.matmul(ps[:m, :], lhsT=xslab[:, off:off + m],
                                 rhs=w1_t[:], start=True, stop=True)
                stg = stg_pool.tile([P, hid], BF16, tag="stg")
                if t % 3 == 2:
                    nc.scalar.activation(stg[:m, :], ps[:m, :], AF.Copy)
                else:
                    nc.vector.tensor_copy(stg[:m, :], ps[:m, :])
                r0 = t * P
                nc.sync.dma_start(out=hs1w[r0:r0 + m, :], in_=stg[:m, :])

        # ------------- aggregation helper (used for both layers) ----------
        def emit_gathers(table_w):
            tiles = []
            for g in range(cfg.ngroups):
                ch = min(cfg.gg, cfg.nchunk - g * cfg.gg)
                gt = g_pool.tile([P, cfg.gg, 2 * hid], BF16, tag="gbuf")
                nc.gpsimd.dma_gather(
                    out_ap=gt[:, :ch, :],
                    in_ap=table_w,
                    idxs_ap=gidx_t[:, g * cfg.gg * 8:(g * cfg.gg + ch) * 8],
                    num_idxs=ch * P,
                    num_idxs_reg=ch * P,
                    elem_size=2 * hid,
                )
                tiles.append(gt)
            return tiles

        def agg_block(b, gtiles):
            """PSUM[hid, 128] = sum over chunks of msg^T contributions."""
            ps = psT.tile([hid, P], F32, tag="psT")
            nch = 2 * cfg.cp
            for j in range(nch):
                g = b * nch + j
                par = j // cfg.cp
                grp, cc = divmod(g, cfg.gg)
                s_t = s_pool.tile([P, P], BF16, tag="smat")
                nc.vector.tensor_scalar(
                    out=s_t[:], in0=iota_t[:], scalar1=dstl_t[:, g:g + 1],
                    scalar2=None, op0=OP.is_equal)
                nc.tensor.matmul(
                    ps[:],
                    lhsT=gtiles[grp][:, cc, par * hid:(par + 1) * hid],
                    rhs=s_t[:], start=(j == 0), stop=(j == nch - 1))
            return ps

        # ---------------- Phase B: layer-1 aggregation -> r1T -------------
        g1 = emit_gathers(hs1.ap())
        for pi, (b0, b1_) in enumerate(cfg.piece_blocks):
            prow0 = sum(cfg.bw[:b0])
            for b in range(b0, b1_):
                bw = cfg.bw[b]
                boff = b * P
                ps = agg_block(b, g1)
                o1 = bpost_pool.tile([hid, P], F32, tag="bpost")
                nc.vector.tensor_tensor(
                    out=o1[:, :bw], in0=ps[:, :bw],
                    in1=dinvT_t[:, boff:boff + bw], op=OP.mult)
                nc.scalar.activation(r1T_sb[:, boff:boff + bw], o1[:, :bw],
                                     AF.Relu, bias=b1_t[:, 0:1])
            rows_p = cfg.piece_rows[pi]
            nc.sync.dma_start(out=r1loc[pi][:, :],
                              in_=r1T_sb[:, prow0:prow0 + rows_p])
            nc.gpsimd.collective_compute(
                "AllGather", OP.bypass,
                replica_groups=[list(range(cfg.ncore))],
                ins=[r1loc[pi].ap().opt()],
                outs=[r1full[pi].ap().opt()],
            )

        # ---------------- Phase C1: hs2 = dinv * (r1 @ W2), replicated ----
        tc_i = 0
        cur = None
        for (pi, rb, t, node0, m) in cfg.c1_tiles:
            rows_p = cfg.piece_rows[pi]
            if cur is None or cur[0] != (pi, rb):
                rbuf = r_pool.tile([hid, max(cfg.piece_rows)], BF16,
                                   tag="rbuf")
                nc.sync.dma_start(out=rbuf[:, :rows_p],
                                  in_=r1full[pi][rb, :, :])
                cur = ((pi, rb), rbuf)
            rbuf = cur[1]
            ps = psA.tile([P, hid], F32, tag="psA")
            nc.tensor.matmul(ps[:m, :], lhsT=rbuf[:, t * P:t * P + m],
                             rhs=w2_t[:], start=True, stop=True)
            stg = stg_pool.tile([P, hid], BF16, tag="stg")
            if tc_i % 3 == 2:
                nc.scalar.activation(stg[:m, :], ps[:m, :], AF.Copy,
                                     scale=dinvc1_t[:m, tc_i:tc_i + 1])
            else:
                nc.vector.tensor_scalar(
                    out=stg[:m, :], in0=ps[:m, :],
                    scalar1=dinvc1_t[:m, tc_i:tc_i + 1], scalar2=None,
                    op0=OP.mult)
            nc.sync.dma_start(out=hs2w[node0:node0 + m, :], in_=stg[:m, :])
            tc_i += 1

        # ---------------- Phase C2: layer-2 aggregation -> softmax --------
        g2 = emit_gathers(hs2.ap())
        for b in range(cfg.nblk):
            bw = cfg.bw[b]
            boff = b * P
            ps = agg_block(b, g2)
            o2 = bpost_pool.tile([hid, P], F32, tag="bpost")
            nc.vector.tensor_tensor(
                out=o2[:, :bw], in0=ps[:, :bw],
                in1=dinvT_t[:, boff:boff + bw], op=OP.mult)
            r2 = bpost_pool.tile([hid + 1, P], BF16, tag="r2")
            nc.scalar.activation(r2[:hid, :bw], o2[:, :bw], AF.Relu,
                                 bias=b2_t[:, 0:1])
            nc.gpsimd.memset(r2[hid:hid + 1, :bw], 1.0)
            pl = psL.tile([P, ncls], F32, tag="psL")
            nc.tensor.matmul(pl[:bw, :], lhsT=r2[:, :bw], rhs=wout_t[:],
                             start=True, stop=True)
            ex = sm_pool.tile([P, ncls], F32, tag="ex")
            nc.scalar.activation(ex[:bw, :], pl[:bw, :], AF.Exp)
            ssum = sm_pool.tile([P, 1], F32, tag="ssum")
            nc.vector.reduce_sum(out=ssum[:bw, :], in_=ex[:bw, :],
                                 axis=mybir.AxisListType.X)
            rec = sm_pool.tile([P, 1], F32, tag="rec")
            nc.vector.reciprocal(rec[:bw, :], ssum[:bw, :])
            ot = sm_pool.tile([P, ncls], F32, tag="ot")
            nc.vector.tensor_scalar(out=ot[:bw, :], in0=ex[:bw, :],
                                    scalar1=rec[:bw, 0:1], scalar2=None,
                                    op0=OP.mult)
            nc.sync.dma_start(out=out[boff:boff + bw, :], in_=ot[:bw, :])

        for p_ in reversed(pools):
            p_.release()

    nc.compile()
    return nc


def run(inputs, trace=False, debug=False, gg=8, npieces=4):
    from concourse.bass_utils import run_bass_kernel_spmd

    cfg, shared, per_core = build_schedule(
        np.asarray(inputs["x"]), np.asarray(inputs["edge_index"]),
        np.asarray(inputs["W1"]), np.asarray(inputs["b1"]),
        np.asarray(inputs["W2"]), np.asarray(inputs["b2"]),
        np.asarray(inputs["Wout"]), np.asarray(inputs["bout"]),
        gg=gg, npieces=npieces)
    nc = build_program(cfg, debug=debug)
    in_maps = [dict(shared, **pc) for pc in per_core]
    res = run_bass_kernel_spmd(nc, in_maps, list(range(cfg.ncore)),
                               trace=trace)
    outs = [res.results[c]["out"] for c in range(cfg.ncore)]
    full = np.concatenate(outs, axis=0).astype(np.float32)
    return full, res, cfg


def kernel(**inputs) -> np.ndarray:
    out, _, _ = run(inputs, trace=False)
    return out


# revision 8
# speedup vs baseline: 1.4647x; 1.4647x over previous
# GCN (2-layer GCNConv + linear head + softmax) on 8 Trainium2 NeuronCores.
#
# Math (matches PyG GCNConv with add_self_loops, symmetric norm):
#   A' = A + I,  deg = indegree(A') ,  dinv = deg^-1/2
#   out = softmax( relu( Ahat @ relu( Ahat @ (x W1) + b1 ) W2 + b2 ) Wout + bout )
#   with Ahat = D^-1/2 A' D^-1/2.
# We push dinv scalings onto node vectors:  Ahat h = dinv * (A'^T-gather-sum (dinv * h)).
#
# Distribution: nodes (rows) are range-sharded across 8 cores; edges are
# partitioned by destination core.  Per destination block of 128 nodes the
# incoming edge list is processed in chunks of 128 edges:
#   gather hs[src] rows with dma_gather (bf16 "pair" table [N/2, 128]: row r
#   holds nodes 2r and 2r+1, 256B — the minimum gather granule),
#   build a one-hot selection matrix S[e, dst] = (iota == dst_local[e]) on DVE,
#   and accumulate aggT[feat, dst] += msg[e, feat]^T via PE matmuls into PSUM.
# Dense phases (x@W1, r1@W2) are computed replicated on every core; the only
# cross-core exchange is an AllGather of r1 (relu of layer-1 output), split
# into pieces so it overlaps the layer-1 aggregation.
import math
import os
import sys
from dataclasses import dataclass, field

import numpy as np

sys.path.insert(0, "/opt/trn_rl_repo")
sys.path.insert(0, "/opt/pypackages")

import ml_dtypes

import concourse.bacc as bacc
import concourse.bass as bass
import concourse.mybir as mybir
import concourse.tile as tile

BF16 = mybir.dt.bfloat16
FP8 = mybir.dt.float8e4
F32 = mybir.dt.float32
I16 = mybir.dt.int16
AF = mybir.ActivationFunctionType
OP = mybir.AluOpType

P = 128


@dataclass
class Cfg:
    n: int            # nodes (even, divisible by ncore)
    ncore: int
    feat: int         # 128
    hid: int          # 64
    ncls: int         # 16
    cp: int           # chunks per (block, parity) bucket  (uniform, SPMD)
    gg: int           # gather group size in chunks
    npieces: int

    ns: int = field(init=False)
    nblk: int = field(init=False)
    bw: list = field(init=False)        # block widths
    npair: int = field(init=False)
    nchunk: int = field(init=False)     # chunks per layer per core
    ngroups: int = field(init=False)
    nt_a: int = field(init=False)       # phase-A tiles over all nodes
    piece_blocks: list = field(init=False)   # list of (b0, b1)
    piece_rows: list = field(init=False)
    c1_tiles: list = field(init=False)  # flat [(piece, rank, t, node0, m)]

    def __post_init__(self):
        self.ns = self.n // self.ncore
        self.nblk = (self.ns + P - 1) // P
        self.bw = [min(P, self.ns - b * P) for b in range(self.nblk)]
        self.npair = self.n // 2
        self.nchunk = self.nblk * 2 * self.cp
        self.ngroups = (self.nchunk + self.gg - 1) // self.gg
        self.nt_a = (self.n + P - 1) // P
        npc = min(self.npieces, self.nblk)
        base, rem = divmod(self.nblk, npc)
        sizes = [base + (1 if i < rem else 0) for i in range(npc)]
        self.piece_blocks = []
        b0 = 0
        for s in sizes:
            self.piece_blocks.append((b0, b0 + s))
            b0 += s
        self.npieces = npc
        self.piece_rows = [
            sum(self.bw[b0:b1]) for (b0, b1) in self.piece_blocks
        ]
        self.c1_tiles = []
        for pi, (b0, b1) in enumerate(self.piece_blocks):
            prow0 = sum(self.bw[:b0])
            rows_p = self.piece_rows[pi]
            ntile = (rows_p + P - 1) // P
            for rb in range(self.ncore):
                for t in range(ntile):
                    m = min(P, rows_p - t * P)
                    node0 = rb * self.ns + prow0 + t * P
                    self.c1_tiles.append((pi, rb, t, node0, m))


def build_schedule(x, edge_index, W1, b1, W2, b2, Wout, bout, ncore=8,
                   npieces=4, gg=8):
    """Host-side preprocessing.  Returns (cfg, shared inputs, per-core inputs)."""
    n, feat = x.shape
    hid = W1.shape[1]
    ncls = Wout.shape[1]
    assert n % (2 * ncore) == 0

    src = np.concatenate([np.asarray(edge_index[0], dtype=np.int64),
                          np.arange(n, dtype=np.int64)]).astype(np.int32)
    dst = np.concatenate([np.asarray(edge_index[1], dtype=np.int64),
                          np.arange(n, dtype=np.int64)]).astype(np.int32)
    deg = np.bincount(dst, minlength=n).astype(np.float64)
    dinv = np.where(deg > 0, 1.0 / np.sqrt(np.maximum(deg, 1e-12)), 0.0)
    dinv = dinv.astype(np.float32)

    ns = n // ncore
    nblk = (ns + P - 1) // P
    core_of = dst // ns
    loc = dst % ns
    blk = loc >> 7
    dstl = (loc & 127).astype(np.float32)
    par = (src & 1).astype(np.int64)
    pidx = (src >> 1).astype(np.int32)

    nbucket = ncore * nblk * 2
    key = (core_of * nblk + blk) * 2 + par
    counts = np.bincount(key, minlength=nbucket)
    cp = int(math.ceil(counts.max() / P))

    cfg = Cfg(n=n, ncore=ncore, feat=feat, hid=hid, ncls=ncls, cp=cp,
              gg=gg, npieces=npieces)

    # Per-edge slot position inside the packed stream of its core.
    order = np.argsort(key, kind="stable")
    bucket_start = np.zeros(nbucket + 1, dtype=np.int64)
    np.cumsum(counts, out=bucket_start[1:])
    rank_in_bucket = np.empty(len(key), dtype=np.int64)
    ar = np.arange(len(key), dtype=np.int64)
    rank_in_bucket[order] = ar - bucket_start[key[order]]
    # slot base of bucket (within its core): (blk*2 + par) * cp * 128
    slot_base = ((blk * 2 + par) * cp) * P
    pos = slot_base + rank_in_bucket  # position within core stream

    tot = cfg.nchunk * P
    gidx_all = []
    dstl_all = []
    dinvT_all = []
    for c in range(ncore):
        sel = core_of == c
        stream_pidx = np.zeros(tot, dtype=np.int16)
        stream_dstl = np.full(tot, -1.0, dtype=np.float32)
        p_c = pos[sel]
        stream_pidx[p_c] = pidx[sel].astype(np.int16)
        stream_dstl[p_c] = dstl[sel]
        # wrapped index layout: idx at linear pos i -> [i % 16, i // 16],
        # replicated over the 8 groups of 16 partitions
        wrapped = stream_pidx.reshape(-1, 16).T  # [16, tot/16]
        gidx = np.tile(wrapped, (8, 1)).astype(np.int16)
        # one-hot S per chunk: smat[p, c*128 + d] = (dstl[c*128+p] == d)
        sm = (stream_dstl.reshape(cfg.nchunk, P)[:, :, None]
              == np.arange(P, dtype=np.float32)[None, None, :])
        sm = sm.transpose(1, 0, 2).reshape(P, cfg.nchunk * P)
        smat = np.ascontiguousarray(sm.astype(ml_dtypes.float8_e4m3fn))
        gidx_all.append(np.ascontiguousarray(gidx))
        dstl_all.append(smat)
        dinvT_all.append(np.ascontiguousarray(
            np.broadcast_to(dinv[c * ns:(c + 1) * ns][None, :], (hid, ns))
        ).astype(np.float32))

    xs = (x.astype(np.float64) * dinv.astype(np.float64)[:, None])
    xsT = np.ascontiguousarray(xs.T.astype(ml_dtypes.bfloat16))

    dinvc1 = np.zeros((P, len(cfg.c1_tiles)), dtype=np.float32)
    for tc, (pi, rb, t, node0, m) in enumerate(cfg.c1_tiles):
        dinvc1[:m, tc] = dinv[node0:node0 + m]

    wout_aug = np.concatenate([Wout.astype(np.float32),
                               bout.astype(np.float32)[None, :]], axis=0)

    shared = {
        "xsT": xsT,
        "w1": np.ascontiguousarray(W1.astype(ml_dtypes.bfloat16)),
        "w2": np.ascontiguousarray(W2.astype(ml_dtypes.bfloat16)),
        "wout": np.ascontiguousarray(wout_aug.astype(ml_dtypes.bfloat16)),
        "b1": np.ascontiguousarray(b1.astype(np.float32)[:, None]),
        "b2": np.ascontiguousarray(b2.astype(np.float32)[:, None]),
        "dinvc1": dinvc1,
    }
    per_core = [
        {"gidx": gidx_all[c], "smat": dstl_all[c], "dinvT": dinvT_all[c]}
        for c in range(ncore)
    ]
    return cfg, shared, per_core


def build_program(cfg: Cfg, debug=False):
    nc = bacc.Bacc("TRN2", debug=debug, enable_asserts=False,
                   target_bir_lowering=False, num_devices=cfg.ncore,
                   num_swdge_queues=4)
    hid, ncls = cfg.hid, cfg.ncls

    xsT = nc.dram_tensor("xsT", [cfg.feat, cfg.n], BF16, kind="ExternalInput")
    gidx = nc.dram_tensor("gidx", [P, cfg.nchunk * 8], I16, kind="ExternalInput")
    smat = nc.dram_tensor("smat", [P, cfg.nchunk * P], FP8, kind="ExternalInput")
    w1 = nc.dram_tensor("w1", [cfg.feat, hid], BF16, kind="ExternalInput")
    w2 = nc.dram_tensor("w2", [hid, hid], BF16, kind="ExternalInput")
    wout = nc.dram_tensor("wout", [hid + 1, ncls], BF16, kind="ExternalInput")
    b1 = nc.dram_tensor("b1", [hid, 1], F32, kind="ExternalInput")
    b2 = nc.dram_tensor("b2", [hid, 1], F32, kind="ExternalInput")
    dinvT = nc.dram_tensor("dinvT", [hid, cfg.ns], F32, kind="ExternalInput")
    dinvc1 = nc.dram_tensor("dinvc1", [P, len(cfg.c1_tiles)], F32,
                            kind="ExternalInput")
    out = nc.dram_tensor("out", [cfg.ns, ncls], F32, kind="ExternalOutput")

    hs1 = nc.dram_tensor("hs1", [cfg.npair, 2 * hid], BF16)
    hs2 = nc.dram_tensor("hs2", [cfg.npair, 2 * hid], BF16)
    r1loc = [nc.dram_tensor(f"r1loc{p}", [hid, cfg.piece_rows[p]], BF16)
             for p in range(cfg.npieces)]
    r1full = [nc.dram_tensor(f"r1full{p}",
                             [cfg.ncore, hid, cfg.piece_rows[p]], BF16,
                             addr_space="Shared")
              for p in range(cfg.npieces)]

    hs1w = hs1.ap().rearrange("a (b c) -> (a b) c", b=2, c=hid)
    hs2w = hs2.ap().rearrange("a (b c) -> (a b) c", b=2, c=hid)

    with tile.TileContext(nc) as tc:
        pools = []

        def mkpool(**kw):
            p = tc.alloc_tile_pool(**kw)
            pools.append(p)
            return p

        cpool = mkpool(name="const", bufs=1)
        w1_t = cpool.tile([cfg.feat, hid], BF16, tag="w1")
        w2_t = cpool.tile([hid, hid], BF16, tag="w2")
        wout_t = cpool.tile([hid + 1, ncls], BF16, tag="wout")
        b1_t = cpool.tile([hid, 1], F32, tag="b1")
        b2_t = cpool.tile([hid, 1], F32, tag="b2")
        dinvT_t = cpool.tile([hid, cfg.ns], F32, tag="dinvT")
        dinvc1_t = cpool.tile([P, len(cfg.c1_tiles)], F32, tag="dinvc1")
        gidx_t = cpool.tile([P, cfg.nchunk * 8], I16, tag="gidx")
        r1T_sb = cpool.tile([hid, cfg.ns], BF16, tag="r1T")

        for t_, d_ in ((w1_t, w1), (w2_t, w2),
                       (wout_t, wout), (b1_t, b1), (b2_t, b2),
                       (dinvT_t, dinvT), (dinvc1_t, dinvc1),
                       (gidx_t, gidx)):
            nc.sync.dma_start(out=t_[:], in_=d_[:, :])

        # pools
        SLAB = 16  # phase-A tiles per slab
        xpool = mkpool(name="xslab", bufs=2)
        stg_pool = mkpool(name="stg", bufs=4)
        s_pool = mkpool(name="smat", bufs=3)
        g_pool = mkpool(name="gbuf", bufs=3)
        r_pool = mkpool(name="rbuf", bufs=2)
        bpost_pool = mkpool(name="bpost", bufs=3)
        sm_pool = mkpool(name="smx", bufs=3)
        psA = mkpool(name="psA", bufs=2, space="PSUM")
        psT = mkpool(name="psT", bufs=2, space="PSUM")
        psL = mkpool(name="psL", bufs=2, space="PSUM")

        # ---------------- Phase A: hs1 = (dinv*x) @ W1, replicated ---------
        nslab = (cfg.nt_a + SLAB - 1) // SLAB
        for s in range(nslab):
            t0 = s * SLAB
            t1 = min(t0 + SLAB, cfg.nt_a)
            c0 = t0 * P
            c1 = min(t1 * P, cfg.n)
            xslab = xpool.tile([cfg.feat, SLAB * P], BF16, tag="xslab")
            nc.sync.dma_start(out=xslab[:, :c1 - c0], in_=xsT[:, c0:c1])
            for t in range(t0, t1):
                m = min(P, cfg.n - t * P)
                off = t * P - c0
                ps = psA.tile([P, hid], F32, tag="psA")
                nc.tensor.matmul(ps[:m, :], lhsT=xslab[:, off:off + m],
                                 rhs=w1_t[:], start=True, stop=True)
                stg = stg_pool.tile([P, hid], BF16, tag="stg")
                if t % 3 == 2:
                    nc.scalar.activation(stg[:m, :], ps[:m, :], AF.Copy)
                else:
                    nc.vector.tensor_copy(stg[:m, :], ps[:m, :])
                r0 = t * P
                nc.sync.dma_start(out=hs1w[r0:r0 + m, :], in_=stg[:m, :])

        # ------------- aggregation helper (used for both layers) ----------
        SSLAB = 16  # chunks of S per DMA slab

        def emit_gathers(table_w):
            tiles = []
            for g in range(cfg.ngroups):
                ch = min(cfg.gg, cfg.nchunk - g * cfg.gg)
                gt = g_pool.tile([P, cfg.gg, 2 * hid], BF16, tag="gbuf")
                nc.gpsimd.dma_gather(
                    out_ap=gt[:, :ch, :],
                    in_ap=table_w,
                    idxs_ap=gidx_t[:, g * cfg.gg * 8:(g * cfg.gg + ch) * 8],
                    num_idxs=ch * P,
                    num_idxs_reg=ch * P,
                    elem_size=2 * hid,
                    queue_num=g % int(os.environ.get("GCN_NQ", "4")),
                )
                tiles.append(gt)
            return tiles

        def emit_sloads():
            tiles = []
            nslab = (cfg.nchunk + SSLAB - 1) // SSLAB
            for g in range(nslab):
                ch = min(SSLAB, cfg.nchunk - g * SSLAB)
                st = s_pool.tile([P, SSLAB * P], FP8, tag="smat")
                nc.sync.dma_start(out=st[:, :ch * P],
                                  in_=smat[:, g * SSLAB * P:(g * SSLAB + ch) * P])
                tiles.append(st)
            return tiles

        def agg_block(b, gtiles, stiles):
            """PSUM[hid, 128] = sum over chunks of msg^T contributions."""
            ps = psT.tile([hid, P], F32, tag="psT")
            nch = 2 * cfg.cp
            for j in range(nch):
                g = b * nch + j
                par = j // cfg.cp
                grp, cc = divmod(g, cfg.gg)
                sg, sc = divmod(g, SSLAB)
                nc.tensor.matmul(
                    ps[:],
                    lhsT=gtiles[grp][:, cc, par * hid:(par + 1) * hid],
                    rhs=stiles[sg][:, sc * P:(sc + 1) * P],
                    start=(j == 0), stop=(j == nch - 1))
            return ps

        # ---------------- Phase B: layer-1 aggregation -> r1T -------------
        g1 = emit_gathers(hs1.ap())
        s1 = emit_sloads()
        for pi, (b0, b1_) in enumerate(cfg.piece_blocks):
            prow0 = sum(cfg.bw[:b0])
            for b in range(b0, b1_):
                bw = cfg.bw[b]
                boff = b * P
                ps = agg_block(b, g1, s1)
                o1 = bpost_pool.tile([hid, P], F32, tag="bpost")
                nc.vector.tensor_tensor(
                    out=o1[:, :bw], in0=ps[:, :bw],
                    in1=dinvT_t[:, boff:boff + bw], op=OP.mult)
                nc.scalar.activation(r1T_sb[:, boff:boff + bw], o1[:, :bw],
                                     AF.Relu, bias=b1_t[:, 0:1])
            rows_p = cfg.piece_rows[pi]
            nc.sync.dma_start(out=r1loc[pi][:, :],
                              in_=r1T_sb[:, prow0:prow0 + rows_p])
            nc.gpsimd.collective_compute(
                "AllGather", OP.bypass,
                replica_groups=[list(range(cfg.ncore))],
                ins=[r1loc[pi].ap().opt()],
                outs=[r1full[pi].ap().opt()],
            )

        # ---------------- Phase C1: hs2 = dinv * (r1 @ W2), replicated ----
        tc_i = 0
        cur = None
        for (pi, rb, t, node0, m) in cfg.c1_tiles:
            rows_p = cfg.piece_rows[pi]
            if cur is None or cur[0] != (pi, rb):
                rbuf = r_pool.tile([hid, max(cfg.piece_rows)], BF16,
                                   tag="rbuf")
                nc.sync.dma_start(out=rbuf[:, :rows_p],
                                  in_=r1full[pi][rb, :, :])
                cur = ((pi, rb), rbuf)
            rbuf = cur[1]
            ps = psA.tile([P, hid], F32, tag="psA")
            nc.tensor.matmul(ps[:m, :], lhsT=rbuf[:, t * P:t * P + m],
                             rhs=w2_t[:], start=True, stop=True)
            stg = stg_pool.tile([P, hid], BF16, tag="stg")
            if tc_i % 3 == 2:
                nc.scalar.activation(stg[:m, :], ps[:m, :], AF.Copy,
                                     scale=dinvc1_t[:m, tc_i:tc_i + 1])
            else:
                nc.vector.tensor_scalar(
                    out=stg[:m, :], in0=ps[:m, :],
                    scalar1=dinvc1_t[:m, tc_i:tc_i + 1], scalar2=None,
                    op0=OP.mult)
            nc.sync.dma_start(out=hs2w[node0:node0 + m, :], in_=stg[:m, :])
            tc_i += 1

        # ---------------- Phase C2: layer-2 aggregation -> softmax --------
        g2 = emit_gathers(hs2.ap())
        s2 = emit_sloads()
        for b in range(cfg.nblk):
            bw = cfg.bw[b]
            boff = b * P
            ps = agg_block(b, g2, s2)
            o2 = bpost_pool.tile([hid, P], F32, tag="bpost")
            nc.vector.tensor_tensor(
                out=o2[:, :bw], in0=ps[:, :bw],
                in1=dinvT_t[:, boff:boff + bw], op=OP.mult)
            r2 = bpost_pool.tile([hid + 1, P], BF16, tag="r2")
            nc.scalar.activation(r2[:hid, :bw], o2[:, :bw], AF.Relu,
                                 bias=b2_t[:, 0:1])
            nc.gpsimd.memset(r2[hid:hid + 1, :bw], 1.0)
            pl = psL.tile([P, ncls], F32, tag="psL")
            nc.tensor.matmul(pl[:bw, :], lhsT=r2[:, :bw], rhs=wout_t[:],
                             start=True, stop=True)
            ex = sm_pool.tile([P, ncls], F32, tag="ex")
            ssum = sm_pool.tile([P, 1], F32, tag="ssum")
            nc.scalar.activation(ex[:bw, :], pl[:bw, :], AF.Exp,
                                 accum_out=ssum[:bw, 0:1])
            rec = sm_pool.tile([P, 1], F32, tag="rec")
            nc.vector.reciprocal(rec[:bw, :], ssum[:bw, :])
            ot = sm_pool.tile([P, ncls], F32, tag="ot")
            nc.vector.tensor_scalar(out=ot[:bw, :], in0=ex[:bw, :],
                                    scalar1=rec[:bw, 0:1], scalar2=None,
                                    op0=OP.mult)
            nc.sync.dma_start(out=out[boff:boff + bw, :], in_=ot[:bw, :])

        for p_ in reversed(pools):
            p_.release()

    nc.compile()
    return nc


def run(inputs, trace=False, debug=False, gg=8, npieces=4):
    from concourse.bass_utils import run_bass_kernel_spmd

    cfg, shared, per_core = build_schedule(
        np.asarray(inputs["x"]), np.asarray(inputs["edge_index"]),
        np.asarray(inputs["W1"]), np.asarray(inputs["b1"]),
        np.asarray(inputs["W2"]), np.asarray(inputs["b2"]),
        np.asarray(inputs["Wout"]), np.asarray(inputs["bout"]),
        gg=gg, npieces=npieces)
    nc = build_program(cfg, debug=debug)
    in_maps = [dict(shared, **pc) for pc in per_core]
    res = run_bass_kernel_spmd(nc, in_maps, list(range(cfg.ncore)),
                               trace=trace)
    outs = [res.results[c]["out"] for c in range(cfg.ncore)]
    full = np.concatenate(outs, axis=0).astype(np.float32)
    return full, res, cfg


def kernel(**inputs) -> np.ndarray:
    out, _, _ = run(inputs, trace=False)
    return out


# revision 11
# speedup vs baseline: 1.8787x; 1.2827x over previous
# GCN (2-layer GCNConv + linear head + softmax) on 8 Trainium2 NeuronCores.
#
# Math (matches PyG GCNConv with add_self_loops, symmetric norm):
#   A' = A + I,  deg = indegree(A') ,  dinv = deg^-1/2
#   out = softmax( relu( Ahat @ relu( Ahat @ (x W1) + b1 ) W2 + b2 ) Wout + bout )
#   with Ahat = D^-1/2 A' D^-1/2.
# We push dinv scalings onto node vectors:  Ahat h = dinv * (A'^T-gather-sum (dinv * h)).
#
# Distribution: nodes (rows) are range-sharded across 8 cores; edges are
# partitioned by destination core.  Per destination block of 128 nodes the
# incoming edge list is processed in chunks of 128 edges:
#   gather hs[src] rows with dma_gather (bf16 "pair" table [N/2, 128]: row r
#   holds nodes 2r and 2r+1, 256B — the minimum gather granule),
#   build a one-hot selection matrix S[e, dst] = (iota == dst_local[e]) on DVE,
#   and accumulate aggT[feat, dst] += msg[e, feat]^T via PE matmuls into PSUM.
# Dense phases (x@W1, r1@W2) are computed replicated on every core; the only
# cross-core exchange is an AllGather of r1 (relu of layer-1 output), split
# into pieces so it overlaps the layer-1 aggregation.
import math
import os
import sys
from dataclasses import dataclass, field

import numpy as np

sys.path.insert(0, "/opt/trn_rl_repo")
sys.path.insert(0, "/opt/pypackages")

import ml_dtypes

import concourse.bacc as bacc
import concourse.bass as bass
import concourse.mybir as mybir
import concourse.tile as tile

BF16 = mybir.dt.bfloat16
FP8 = mybir.dt.float8e4
F32 = mybir.dt.float32
I16 = mybir.dt.int16
AF = mybir.ActivationFunctionType
OP = mybir.AluOpType

P = 128


@dataclass
class Cfg:
    n: int            # nodes (even, divisible by ncore)
    ncore: int
    feat: int         # 128
    hid: int          # 64
    ncls: int         # 16
    cp: int           # chunks per (block, parity) bucket  (uniform, SPMD)
    gg: int           # gather group size in chunks
    npieces: int

    ns: int = field(init=False)
    nblk: int = field(init=False)
    bw: list = field(init=False)        # block widths
    npair: int = field(init=False)
    nchunk: int = field(init=False)     # chunks per layer per core
    ngroups: int = field(init=False)
    nt_a: int = field(init=False)       # phase-A tiles over all nodes
    piece_blocks: list = field(init=False)   # list of (b0, b1)
    piece_rows: list = field(init=False)
    c1_tiles: list = field(init=False)  # flat [(piece, rank, t, node0, m)]

    def __post_init__(self):
        self.ns = self.n // self.ncore
        self.nblk = (self.ns + P - 1) // P
        self.bw = [min(P, self.ns - b * P) for b in range(self.nblk)]
        self.npair = self.n // 2
        self.nchunk = self.nblk * 2 * self.cp
        self.ngroups = (self.nchunk + self.gg - 1) // self.gg
        self.nt_a = (self.n + P - 1) // P
        npc = min(self.npieces, self.nblk)
        base, rem = divmod(self.nblk, npc)
        sizes = [base + (1 if i < rem else 0) for i in range(npc)]
        self.piece_blocks = []
        b0 = 0
        for s in sizes:
            self.piece_blocks.append((b0, b0 + s))
            b0 += s
        self.npieces = npc
        self.piece_rows = [
            sum(self.bw[b0:b1]) for (b0, b1) in self.piece_blocks
        ]
        self.c1_tiles = []
        for pi, (b0, b1) in enumerate(self.piece_blocks):
            prow0 = sum(self.bw[:b0])
            rows_p = self.piece_rows[pi]
            ntile = (rows_p + P - 1) // P
            for rb in range(self.ncore):
                for t in range(ntile):
                    m = min(P, rows_p - t * P)
                    node0 = rb * self.ns + prow0 + t * P
                    self.c1_tiles.append((pi, rb, t, node0, m))


def build_schedule(x, edge_index, W1, b1, W2, b2, Wout, bout, ncore=8,
                   npieces=4, gg=8):
    """Host-side preprocessing.  Returns (cfg, shared inputs, per-core inputs)."""
    n, feat = x.shape
    hid = W1.shape[1]
    ncls = Wout.shape[1]
    assert n % (2 * ncore) == 0

    src = np.concatenate([np.asarray(edge_index[0], dtype=np.int64),
                          np.arange(n, dtype=np.int64)]).astype(np.int32)
    dst = np.concatenate([np.asarray(edge_index[1], dtype=np.int64),
                          np.arange(n, dtype=np.int64)]).astype(np.int32)
    deg = np.bincount(dst, minlength=n).astype(np.float64)
    dinv = np.where(deg > 0, 1.0 / np.sqrt(np.maximum(deg, 1e-12)), 0.0)
    dinv = dinv.astype(np.float32)

    ns = n // ncore
    nblk = (ns + P - 1) // P
    core_of = dst // ns
    loc = dst % ns
    blk = loc >> 7
    dstl = (loc & 127).astype(np.float32)
    par = (src & 1).astype(np.int64)
    pidx = (src >> 1).astype(np.int32)

    nbucket = ncore * nblk * 2
    key = (core_of * nblk + blk) * 2 + par
    counts = np.bincount(key, minlength=nbucket)
    cp = int(math.ceil(counts.max() / P))

    cfg = Cfg(n=n, ncore=ncore, feat=feat, hid=hid, ncls=ncls, cp=cp,
              gg=gg, npieces=npieces)

    # Per-edge slot position inside the packed stream of its core.
    order = np.argsort(key, kind="stable")
    bucket_start = np.zeros(nbucket + 1, dtype=np.int64)
    np.cumsum(counts, out=bucket_start[1:])
    rank_in_bucket = np.empty(len(key), dtype=np.int64)
    ar = np.arange(len(key), dtype=np.int64)
    rank_in_bucket[order] = ar - bucket_start[key[order]]
    # slot base of bucket (within its core): (blk*2 + par) * cp * 128
    slot_base = ((blk * 2 + par) * cp) * P
    pos = slot_base + rank_in_bucket  # position within core stream

    tot = cfg.nchunk * P
    gidx_all = []
    dstl_all = []
    dinvT_all = []
    for c in range(ncore):
        sel = core_of == c
        stream_pidx = np.zeros(tot, dtype=np.int16)
        stream_dstl = np.full(tot, -1.0, dtype=np.float32)
        p_c = pos[sel]
        stream_pidx[p_c] = pidx[sel].astype(np.int16)
        stream_dstl[p_c] = dstl[sel]
        # wrapped index layout: idx at linear pos i -> [i % 16, i // 16],
        # replicated over the 8 groups of 16 partitions
        wrapped = stream_pidx.reshape(-1, 16).T  # [16, tot/16]
        gidx = np.tile(wrapped, (8, 1)).astype(np.int16)
        # one-hot S per chunk: smat[p, c*128 + d] = (dstl[c*128+p] == d)
        sm = (stream_dstl.reshape(cfg.nchunk, P)[:, :, None]
              == np.arange(P, dtype=np.float32)[None, None, :])
        sm = sm.transpose(1, 0, 2).reshape(P, cfg.nchunk * P)
        smat = np.ascontiguousarray(sm.astype(ml_dtypes.float8_e4m3fn))
        gidx_all.append(np.ascontiguousarray(gidx))
        dstl_all.append(smat)
        dinvT_all.append(np.ascontiguousarray(
            np.broadcast_to(dinv[c * ns:(c + 1) * ns][None, :], (hid, ns))
        ).astype(np.float32))

    xs = (x.astype(np.float64) * dinv.astype(np.float64)[:, None])
    xsT = np.ascontiguousarray(xs.T.astype(ml_dtypes.bfloat16))

    dinvc1 = np.zeros((P, len(cfg.c1_tiles)), dtype=np.float32)
    for tc, (pi, rb, t, node0, m) in enumerate(cfg.c1_tiles):
        dinvc1[:m, tc] = dinv[node0:node0 + m]

    wout_aug = np.concatenate([Wout.astype(np.float32),
                               bout.astype(np.float32)[None, :]], axis=0)
    ident = np.eye(P, dtype=np.float32).astype(ml_dtypes.bfloat16)

    shared = {
        "xsT": xsT,
        "w1": np.ascontiguousarray(W1.astype(ml_dtypes.bfloat16)),
        "w2": np.ascontiguousarray(W2.astype(ml_dtypes.bfloat16)),
        "wout": np.ascontiguousarray(wout_aug.astype(ml_dtypes.bfloat16)),
        "b1": np.ascontiguousarray(b1.astype(np.float32)[:, None]),
        "ident": np.ascontiguousarray(ident),
        "b2": np.ascontiguousarray(b2.astype(np.float32)[:, None]),
        "dinvc1": dinvc1,
    }
    per_core = [
        {"gidx": gidx_all[c], "smat": dstl_all[c], "dinvT": dinvT_all[c]}
        for c in range(ncore)
    ]
    return cfg, shared, per_core


def build_program(cfg: Cfg, debug=False):
    nc = bacc.Bacc("TRN2", debug=debug, enable_asserts=False,
                   target_bir_lowering=False, num_devices=cfg.ncore,
                   num_swdge_queues=4)
    hid, ncls = cfg.hid, cfg.ncls

    xsT = nc.dram_tensor("xsT", [cfg.feat, cfg.n], BF16, kind="ExternalInput")
    gidx = nc.dram_tensor("gidx", [P, cfg.nchunk * 8], I16, kind="ExternalInput")
    smat = nc.dram_tensor("smat", [P, cfg.nchunk * P], FP8, kind="ExternalInput")
    w1 = nc.dram_tensor("w1", [cfg.feat, hid], BF16, kind="ExternalInput")
    w2 = nc.dram_tensor("w2", [hid, hid], BF16, kind="ExternalInput")
    wout = nc.dram_tensor("wout", [hid + 1, ncls], BF16, kind="ExternalInput")
    b1 = nc.dram_tensor("b1", [hid, 1], F32, kind="ExternalInput")
    ident = nc.dram_tensor("ident", [P, P], BF16, kind="ExternalInput")
    b2 = nc.dram_tensor("b2", [hid, 1], F32, kind="ExternalInput")
    dinvT = nc.dram_tensor("dinvT", [hid, cfg.ns], F32, kind="ExternalInput")
    dinvc1 = nc.dram_tensor("dinvc1", [P, len(cfg.c1_tiles)], F32,
                            kind="ExternalInput")
    out = nc.dram_tensor("out", [cfg.ns, ncls], F32, kind="ExternalOutput")

    hs1 = nc.dram_tensor("hs1", [cfg.npair, 2 * hid], BF16)
    hs2 = nc.dram_tensor("hs2", [cfg.npair, 2 * hid], BF16)
    r1loc = [nc.dram_tensor(f"r1loc{p}", [hid, cfg.piece_rows[p]], BF16)
             for p in range(cfg.npieces)]
    r1full = [nc.dram_tensor(f"r1full{p}",
                             [cfg.ncore, hid, cfg.piece_rows[p]], BF16,
                             addr_space="Shared")
              for p in range(cfg.npieces)]

    hs1w = hs1.ap().rearrange("a (b c) -> (a b) c", b=2, c=hid)
    hs2w = hs2.ap().rearrange("a (b c) -> (a b) c", b=2, c=hid)

    with tile.TileContext(nc) as tc:
        pools = []

        def mkpool(**kw):
            p = tc.alloc_tile_pool(**kw)
            pools.append(p)
            return p

        cpool = mkpool(name="const", bufs=1)
        w1_t = cpool.tile([cfg.feat, hid], BF16, tag="w1")
        w2_t = cpool.tile([hid, hid], BF16, tag="w2")
        wout_t = cpool.tile([hid + 1, ncls], BF16, tag="wout")
        b1_t = cpool.tile([hid, 1], F32, tag="b1")
        ident_t = cpool.tile([P, P], BF16, tag="ident")
        b2_t = cpool.tile([hid, 1], F32, tag="b2")
        dinvT_t = cpool.tile([hid, cfg.ns], F32, tag="dinvT")
        dinvc1_t = cpool.tile([P, len(cfg.c1_tiles)], F32, tag="dinvc1")
        gidx_t = cpool.tile([P, cfg.nchunk * 8], I16, tag="gidx")
        r1T_p = [cpool.tile([hid, cfg.piece_rows[pi]], BF16, tag=f"r1T{pi}",
                            name=f"r1T{pi}")
                 for pi in range(cfg.npieces)]

        for t_, d_ in ((w1_t, w1), (ident_t, ident), (w2_t, w2),
                       (wout_t, wout), (b1_t, b1), (b2_t, b2),
                       (dinvT_t, dinvT), (dinvc1_t, dinvc1),
                       (gidx_t, gidx)):
            nc.sync.dma_start(out=t_[:], in_=d_[:, :])

        # pools
        SLAB = 16  # phase-A tiles per slab
        xpool = mkpool(name="xslab", bufs=2)
        stg_pool = mkpool(name="stg", bufs=4)
        s_pool = mkpool(name="smat", bufs=3)
        g_pool = mkpool(name="gbuf", bufs=3)
        r_pool = mkpool(name="rbuf", bufs=2)
        bpost_pool = mkpool(name="bpost", bufs=3)
        sm_pool = mkpool(name="smx", bufs=3)
        psA = mkpool(name="psA", bufs=2, space="PSUM")
        psT = mkpool(name="psT", bufs=2, space="PSUM")
        psT2 = mkpool(name="psT2", bufs=2, space="PSUM")
        psL = mkpool(name="psL", bufs=2, space="PSUM")

        # ---------------- Phase A: hs1 = (dinv*x) @ W1, replicated ---------
        KB = 8  # tiles per batched hs write
        nslab = (cfg.nt_a + SLAB - 1) // SLAB
        for s in range(nslab):
            t0 = s * SLAB
            t1 = min(t0 + SLAB, cfg.nt_a)
            c0 = t0 * P
            c1 = min(t1 * P, cfg.n)
            xslab = xpool.tile([cfg.feat, SLAB * P], BF16, tag="xslab")
            nc.sync.dma_start(out=xslab[:, :c1 - c0], in_=xsT[:, c0:c1])
            t = t0
            while t < t1:
                kb = min(KB, t1 - t)
                if t + kb > 390:  # keep the tail tile on its own
                    kb = max(1, 390 - t) if t < 390 else 1
                full = all(min(P, cfg.n - (t + k) * P) == P for k in range(kb))
                if not full:
                    kb = 1
                stg = stg_pool.tile([P, KB * hid], BF16, tag="stg")
                for k in range(kb):
                    tt = t + k
                    m = min(P, cfg.n - tt * P)
                    off = tt * P - c0
                    ps = psA.tile([P, hid], F32, tag="psA")
                    nc.tensor.matmul(ps[:m, :], lhsT=xslab[:, off:off + m],
                                     rhs=w1_t[:], start=True, stop=True)
                    if tt % 3 == 2:
                        nc.scalar.activation(stg[:m, k * hid:(k + 1) * hid],
                                             ps[:m, :], AF.Copy)
                    else:
                        nc.vector.tensor_copy(stg[:m, k * hid:(k + 1) * hid],
                                              ps[:m, :])
                r0 = t * P
                m_last = min(P, cfg.n - (t + kb - 1) * P)
                rows = (kb - 1) * P + m_last
                dst_ap = hs1w[r0:r0 + rows, :]
                if kb > 1:
                    dst_ap = dst_ap.rearrange("(k p) f -> p k f", p=P)
                    src_ap = stg[:].rearrange("p (k f) -> p k f", f=hid)[:, :kb, :]
                else:
                    src_ap = stg[:m_last, 0:hid]
                eng = nc.sync if (t // KB) % 2 == 0 else nc.scalar
                eng.dma_start(out=dst_ap, in_=src_ap)
                t += kb

        # ------------- aggregation helper (used for both layers) ----------
        SSLAB = 16  # chunks of S per DMA slab

        def emit_gathers(table_w):
            tiles = []
            for g in range(cfg.ngroups):
                ch = min(cfg.gg, cfg.nchunk - g * cfg.gg)
                gt = g_pool.tile([P, cfg.gg, 2 * hid], BF16, tag="gbuf")
                nc.gpsimd.dma_gather(
                    out_ap=gt[:, :ch, :],
                    in_ap=table_w,
                    idxs_ap=gidx_t[:, g * cfg.gg * 8:(g * cfg.gg + ch) * 8],
                    num_idxs=ch * P,
                    num_idxs_reg=ch * P,
                    elem_size=2 * hid,
                    queue_num=g % int(os.environ.get("GCN_NQ", "4")),
                )
                tiles.append(gt)
            return tiles

        def emit_sloads():
            tiles = []
            nslab = (cfg.nchunk + SSLAB - 1) // SSLAB
            for g in range(nslab):
                ch = min(SSLAB, cfg.nchunk - g * SSLAB)
                st = s_pool.tile([P, SSLAB * P], FP8, tag="smat")
                nc.sync.dma_start(out=st[:, :ch * P],
                                  in_=smat[:, g * SSLAB * P:(g * SSLAB + ch) * P])
                tiles.append(st)
            return tiles

        def agg_block(b, gtiles, stiles):
            """agg[128 dst, hid] in PSUM, then PE-transpose to [hid, 128]."""
            ps = psT.tile([P, hid], F32, tag="psT")
            nch = 2 * cfg.cp
            for j in range(nch):
                g = b * nch + j
                par = j // cfg.cp
                grp, cc = divmod(g, cfg.gg)
                sg, sc = divmod(g, SSLAB)
                nc.tensor.matmul(
                    ps[:],
                    lhsT=stiles[sg][:, sc * P:(sc + 1) * P],
                    rhs=gtiles[grp][:, cc, par * hid:(par + 1) * hid],
                    start=(j == 0), stop=(j == nch - 1))
            # transpose agg -> [hid, 128] via PE (matmul with identity rhs)
            atmp = bpost_pool.tile([P, hid], BF16, tag="atmp")
            nc.vector.tensor_copy(atmp[:], ps[:])
            psx = psT2.tile([hid, P], F32, tag="psT2")
            nc.tensor.matmul(psx[:], lhsT=atmp[:], rhs=ident_t[:],
                             start=True, stop=True)
            return psx

        # ---------------- Phase B: layer-1 aggregation -> r1T -------------
        g1 = emit_gathers(hs1.ap())
        s1 = emit_sloads()
        for pi, (b0, b1_) in enumerate(cfg.piece_blocks):
            prow0 = sum(cfg.bw[:b0])
            for b in range(b0, b1_):
                bw = cfg.bw[b]
                boff = b * P
                ps = agg_block(b, g1, s1)
                o1 = bpost_pool.tile([hid, P], F32, tag="bpost")
                nc.vector.tensor_tensor(
                    out=o1[:, :bw], in0=ps[:, :bw],
                    in1=dinvT_t[:, boff:boff + bw], op=OP.mult)
                nc.scalar.activation(
                    r1T_p[pi][:, boff - prow0:boff - prow0 + bw], o1[:, :bw],
                    AF.Relu, bias=b1_t[:, 0:1])
            rows_p = cfg.piece_rows[pi]
            nc.sync.dma_start(out=r1loc[pi][:, :], in_=r1T_p[pi][:, :])
            nc.gpsimd.collective_compute(
                "AllGather", OP.bypass,
                replica_groups=[list(range(cfg.ncore))],
                ins=[r1loc[pi].ap().opt()],
                outs=[r1full[pi].ap().opt()],
            )

        # ---------------- Phase C1: hs2 = dinv * (r1 @ W2), replicated ----
        from itertools import groupby
        c1_seq = list(enumerate(cfg.c1_tiles))
        gi_ = 0
        for (pi, rb), grp_iter in groupby(c1_seq, key=lambda e: (e[1][0], e[1][1])):
            grp = list(grp_iter)
            rows_p = cfg.piece_rows[pi]
            rbuf = r_pool.tile([hid, max(cfg.piece_rows)], BF16, tag="rbuf")
            nc.sync.dma_start(out=rbuf[:, :rows_p], in_=r1full[pi][rb, :, :])
            i = 0
            while i < len(grp):
                kb = 1
                while (kb < KB and i + kb < len(grp)
                       and grp[i + kb - 1][1][4] == P):
                    kb += 1
                if grp[i + kb - 1][1][4] != P and kb > 1:
                    kb -= 1
                stg = stg_pool.tile([P, KB * hid], BF16, tag="stg")
                for k in range(kb):
                    tc_i, (pi_, rb_, t, node0, m) = grp[i + k]
                    ps = psA.tile([P, hid], F32, tag="psA")
                    nc.tensor.matmul(ps[:m, :], lhsT=rbuf[:, t * P:t * P + m],
                                     rhs=w2_t[:], start=True, stop=True)
                    if tc_i % 3 == 2:
                        nc.scalar.activation(
                            stg[:m, k * hid:(k + 1) * hid], ps[:m, :], AF.Copy,
                            scale=dinvc1_t[:m, tc_i:tc_i + 1])
                    else:
                        nc.vector.tensor_scalar(
                            out=stg[:m, k * hid:(k + 1) * hid], in0=ps[:m, :],
                            scalar1=dinvc1_t[:m, tc_i:tc_i + 1], scalar2=None,
                            op0=OP.mult)
                node0 = grp[i][1][3]
                m_last = grp[i + kb - 1][1][4]
                rows = (kb - 1) * P + m_last
                dst_ap = hs2w[node0:node0 + rows, :]
                if kb > 1:
                    dst_ap = dst_ap.rearrange("(k p) f -> p k f", p=P)
                    src_ap = stg[:].rearrange("p (k f) -> p k f", f=hid)[:, :kb, :]
                else:
                    src_ap = stg[:m_last, 0:hid]
                eng = nc.sync if gi_ % 2 == 0 else nc.scalar
                eng.dma_start(out=dst_ap, in_=src_ap)
                gi_ += 1
                i += kb

        # ---------------- Phase C2: layer-2 aggregation -> softmax --------
        g2 = emit_gathers(hs2.ap())
        s2 = emit_sloads()
        for b in range(cfg.nblk):
            bw = cfg.bw[b]
            boff = b * P
            ps = agg_block(b, g2, s2)
            o2 = bpost_pool.tile([hid, P], F32, tag="bpost")
            nc.vector.tensor_tensor(
                out=o2[:, :bw], in0=ps[:, :bw],
                in1=dinvT_t[:, boff:boff + bw], op=OP.mult)
            r2 = bpost_pool.tile([hid + 1, P], BF16, tag="r2")
            nc.scalar.activation(r2[:hid, :bw], o2[:, :bw], AF.Relu,
                                 bias=b2_t[:, 0:1])
            nc.vector.memset(r2[hid:hid + 1, :bw], 1.0)
            pl = psL.tile([P, ncls], F32, tag="psL")
            nc.tensor.matmul(pl[:bw, :], lhsT=r2[:, :bw], rhs=wout_t[:],
                             start=True, stop=True)
            ex = sm_pool.tile([P, ncls], F32, tag="ex")
            ssum = sm_pool.tile([P, 1], F32, tag="ssum")
            nc.scalar.activation(ex[:bw, :], pl[:bw, :], AF.Exp,
                                 accum_out=ssum[:bw, 0:1])
            rec = sm_pool.tile([P, 1], F32, tag="rec")
            nc.vector.reciprocal(rec[:bw, :], ssum[:bw, :])
            ot = sm_pool.tile([P, ncls], F32, tag="ot")
            nc.vector.tensor_scalar(out=ot[:bw, :], in0=ex[:bw, :],
                                    scalar1=rec[:bw, 0:1], scalar2=None,
                                    op0=OP.mult)
            nc.sync.dma_start(out=out[boff:boff + bw, :], in_=ot[:bw, :])

        for p_ in reversed(pools):
            p_.release()

    nc.compile()
    return nc


def run(inputs, trace=False, debug=False, gg=8, npieces=4):
    from concourse.bass_utils import run_bass_kernel_spmd

    cfg, shared, per_core = build_schedule(
        np.asarray(inputs["x"]), np.asarray(inputs["edge_index"]),
        np.asarray(inputs["W1"]), np.asarray(inputs["b1"]),
        np.asarray(inputs["W2"]), np.asarray(inputs["b2"]),
        np.asarray(inputs["Wout"]), np.asarray(inputs["bout"]),
        gg=gg, npieces=npieces)
    nc = build_program(cfg, debug=debug)
    in_maps = [dict(shared, **pc) for pc in per_core]
    res = run_bass_kernel_spmd(nc, in_maps, list(range(cfg.ncore)),
                               trace=trace)
    outs = [res.results[c]["out"] for c in range(cfg.ncore)]
    full = np.concatenate(outs, axis=0).astype(np.float32)
    return full, res, cfg


def kernel(**inputs) -> np.ndarray:
    out, _, _ = run(inputs, trace=False)
    return out
